# revision 16
# baseline (speedup 1.0000x reference)
"""Gaussian KDE (bandwidth=0.5) on 8 TRN2 NeuronCores — grid-factorized.

out[j] = sum_i mask_i * exp(-|s_i - l_j|^2 / bw^2), normalized to sum 1.

Algorithm (exact Gaussian-lattice factorization, NOT an approximation knob):
  exp(-|s-l|^2/(2v)) with v = bw^2/2 = 0.125 factorizes over a uniform grid
  g_u = h*c_u (c_u = u-63.5, h = 2M/119, M = per-axis abs-max of locations):

      sum_u exp(-(s-g_u)^2/(2h^2)) * exp(-(g_u-l)^2/(2v'))
        = C * exp(-(s-l)^2/(2(v'+h^2)))        [Gaussian o Gaussian, exact]
  with v' = v - h^2.  The lattice-sum constant C is independent of s up to
  a Poisson ripple exp(-2 pi^2) ~ 5e-9, and cancels in the normalization.

  So per core (samples sharded 8-way, locations sharded 8-way):
    Wx[i,u] = exp(-(sx_i-g_u)^2/(2h^2))   (x-window), same Wy     [2048 x 128]
    Ht[v,u] = sum_i Wy[i,v]*Wx[i,u]       (PE, partial over sample shard)
    P[j,u]  = exp(-a'(gx_u-lx_j)^2), Q[j,v] = exp(-a'(gy_v-ly_j)^2),
              a' = 1/(2 v')               (location shard, 1024 locs)
    T2[v,u] = sum_j Q[j,v]*P[j,u]         (PE, partial over location shard)
    ONE AllReduce of [Ht | T2]  (128x256 f32)
    R[j,u]  = sum_v Qt[v,j]*Ht[v,u]       (PE)
    out[j]  = sum_u P[j,u]*R[j,u],  norm = sum_{v,u} Ht*T2  (= sum_j out_j)
    out /= norm  (on device)

  Samples outside the location bbox (strict |s| < M per axis, torch mask
  semantics) are pushed +1000 before binning -> their window underflows to 0.

Engine plan: ScalarE runs ONLY Exp (no act-table switches); DVE+GpSimd build
the quadratic exp arguments with tensor_scalar/scalar_tensor_tensor; PE does
the three contractions in bf16 (operands are exps in [0,1]; rel err ~1e-3).
"""

import sys

sys.path.insert(0, "/opt/trn_rl_repo")

import numpy as np

N_CORES = 8
NS = 16384
NL = 8192
NS_SH = NS // N_CORES  # 2048 samples per core
NL_SH = NL // N_CORES  # 1024 locations per core
G = 128  # grid nodes per axis
NSB = NS_SH // 128  # 16 sample blocks
NLB = NL_SH // 128  # 8 location blocks
GDEN = 119.0  # grid half-width = M * 127/119ish margin (4h pad for windows)
V = 0.125  # bw^2 / 2

_STATE = {}


def build_nc():
    import concourse.bacc as bacc
    import concourse.mybir as mybir
    import concourse.tile as tile
    from concourse import bass_isa

    f32 = mybir.dt.float32
    bf16 = mybir.dt.bfloat16
    AX = mybir.AxisListType
    AF = mybir.ActivationFunctionType
    AL = mybir.AluOpType
    RO = bass_isa.ReduceOp

    nc = bacc.Bacc(None, target_bir_lowering=False, num_devices=N_CORES)

    s_cols = nc.declare_dram_parameter("s_cols", [128, 2 * NSB], f32, isOutput=False)
    l_xc = nc.declare_dram_parameter("l_xcols", [128, NLB], f32, isOutput=False)
    l_yc = nc.declare_dram_parameter("l_ycols", [128, NLB], f32, isOutput=False)
    l_yr = nc.declare_dram_parameter("l_yrow", [1, NL_SH], f32, isOutput=False)
    l_all = nc.declare_dram_parameter("l_all", [128, 128], f32, isOutput=False)
    iot_d = nc.declare_dram_parameter("iota_cb", [128, 2 * G], f32, isOutput=False)
    col_d = nc.declare_dram_parameter("colc", [128, 1], f32, isOutput=False)
    out_d = nc.declare_dram_parameter("out", [128, NLB], f32, isOutput=True)

    with tile.TileContext(nc) as tc:
        with tc.tile_pool(name="const", bufs=1) as cpool, \
             tc.tile_pool(name="dram", bufs=1, space="DRAM") as dpool, \
             tc.tile_pool(name="wa", bufs=3) as wapool, \
             tc.tile_pool(name="wexp", bufs=4) as wepool, \
             tc.tile_pool(name="ps", bufs=1, space="PSUM") as ppool:

            SC = cpool.tile([128, 2 * NSB], f32)  # sample cols [sx | sy]
            LXC = cpool.tile([128, NLB], f32)
            LYC = cpool.tile([128, NLB], f32)
            LYR = cpool.tile([1, NL_SH], f32)
            LA = cpool.tile([128, 128], f32)
            IOT = cpool.tile([128, 2 * G], f32)  # c_u both halves
            COLC = cpool.tile([128, 1], f32)  # c_p per partition

            rm = cpool.tile([128, 2], f32)
            Mb = cpool.tile([128, 2], f32)
            h = cpool.tile([128, 2], f32)
            rh = cpool.tile([128, 2], f32)
            hsq = cpool.tile([128, 2], f32)
            vp = cpool.tile([128, 2], f32)
            rvp = cpool.tile([128, 2], f32)
            na = cpool.tile([128, 2], f32)  # -a' per axis
            gqc = cpool.tile([128, 1], f32)  # gy_v = h_y * c_v

            nSC = cpool.tile([128, 2 * NSB], f32)
            U4 = cpool.tile([128, 4 * NSB], f32)
            Ux = cpool.tile([128, NSB], f32)
            Uy = cpool.tile([128, NSB], f32)
            msk = cpool.tile([128, NSB], f32)
            pm = cpool.tile([128, NSB], f32)
            spx = cpool.tile([128, NSB], f32)
            spy = cpool.tile([128, NSB], f32)
            zx = cpool.tile([128, NSB], f32)
            zy = cpool.tile([128, NSB], f32)

            GP = cpool.tile([128, 2 * G], f32)  # [gx_u | gy_u]
            LYB = cpool.tile([128, NL_SH], f32)
            QD = cpool.tile([128, NL_SH], f32)
            QS = cpool.tile([128, NL_SH], f32)
            Qt = cpool.tile([128, NL_SH], bf16)
            PQE = [cpool.tile([128, 2 * G], f32, name=f"pqe{q}") for q in range(NLB)]

            CCS = cpool.tile([128, 2 * G], f32)
            Hb = cpool.tile([128, G], bf16)
            ONEC = cpool.tile([128, 1], f32)
            ONER = cpool.tile([1, 128], f32)
            rtot_sb = cpool.tile([1, 1], f32)
            rb_sb = cpool.tile([128, 1], f32)
            HTg = cpool.tile([128, 2 * G], f32)
            ACC = cpool.tile([128, NLB], f32)
            scr = cpool.tile([128, G], f32)
            scr2 = cpool.tile([128, G], f32)
            ns_ = cpool.tile([128, 1], f32)
            ntb = cpool.tile([128, 1], f32)
            rtot = cpool.tile([128, 1], f32)
            OUT = cpool.tile([128, NLB], f32)

            cc_in = dpool.tile([128, 2 * G], f32, name="cc_in")
            cc_out = dpool.tile([128, 2 * G], f32, addr_space="Shared", name="cc_out")

            Ht_ps = ppool.tile([128, G], f32, tag="ht")
            T2_ps = ppool.tile([128, G], f32, tag="t2")
            R_ps = ppool.tile([128, NL_SH], f32, tag="r")

            # ---- cc-stream warmup: tiny dummy AR so the real one pays no
            # trigger-start delay (the first collective also absorbs the
            # comm-init barrier wait) ----
            DUM = cpool.tile([1, 4], f32)
            dum_in = dpool.tile([1, 4], f32, name="dum_in")
            dum_out = dpool.tile([1, 4], f32, addr_space="Shared", name="dum_out")
            nc.gpsimd.memset(DUM[:, :], 0.0)
            nc.sync.dma_start(out=dum_in[:, :], in_=DUM[:, :])
            nc.gpsimd.collective_compute(
                "AllReduce",
                AL.add,
                replica_groups=[list(range(N_CORES))],
                ins=[dum_in[:, :]],
                outs=[dum_out[:, :]],
            )

            # ---- input loads ----
            nc.sync.dma_start(out=SC[:, :], in_=s_cols[:, :])
            nc.sync.dma_start(out=LXC[:, :], in_=l_xc[:, :])
            nc.sync.dma_start(out=LYC[:, :], in_=l_yc[:, :])
            nc.sync.dma_start(out=LYR[:, :], in_=l_yr[:, :])
            nc.sync.dma_start(out=LA[:, :], in_=l_all[:, :])
            nc.sync.dma_start(out=IOT[:, :], in_=iot_d[:, :])
            nc.sync.dma_start(out=COLC[:, :], in_=col_d[:, :])

            # ---- bbox bounds M (global over all 8192 locations) ----
            nc.vector.tensor_reduce(
                rm[:, 0:1], LA[:, 0:64], axis=AX.X, op=AL.max,
                apply_absolute_value=True,
            )
            nc.vector.tensor_reduce(
                rm[:, 1:2], LA[:, 64:128], axis=AX.X, op=AL.max,
                apply_absolute_value=True,
            )
            nc.gpsimd.partition_all_reduce(Mb[:, :], rm[:, :], 128, RO.max)

            # ---- runtime scalars (all [128,2] broadcast, x col 0 / y col 1) ----
            nc.vector.tensor_scalar(h[:], Mb[:], 2.0 / GDEN, None, AL.mult)
            nc.vector.reciprocal(rh[:], h[:])
            nc.vector.tensor_tensor(hsq[:], h[:], h[:], AL.mult)
            nc.vector.tensor_scalar(vp[:], hsq[:], -1.0, V, AL.mult, AL.add)
            nc.vector.reciprocal(rvp[:], vp[:])
            nc.vector.tensor_scalar(na[:], rvp[:], -0.5, None, AL.mult)
            nc.vector.tensor_scalar(gqc[:], COLC[:], h[:, 1:2], None, AL.mult)

            # ---- sample prep: mask + z = s/h  ([128, NSB] col k = block) ----
            nc.vector.tensor_scalar(nSC[:], SC[:], -1.0, None, AL.mult)
            nc.vector.tensor_scalar(
                U4[:, 0:NSB], SC[:, 0:NSB], Mb[:, 0:1], None, AL.is_lt
            )
            nc.vector.tensor_scalar(
                U4[:, NSB : 2 * NSB], nSC[:, 0:NSB], Mb[:, 0:1], None, AL.is_lt
            )
            nc.vector.tensor_scalar(
                U4[:, 2 * NSB : 3 * NSB], SC[:, NSB : 2 * NSB], Mb[:, 1:2], None,
                AL.is_lt,
            )
            nc.vector.tensor_scalar(
                U4[:, 3 * NSB : 4 * NSB], nSC[:, NSB : 2 * NSB], Mb[:, 1:2], None,
                AL.is_lt,
            )
            nc.vector.tensor_tensor(
                Ux[:], U4[:, 0:NSB], U4[:, NSB : 2 * NSB], AL.mult
            )
            nc.vector.tensor_tensor(
                Uy[:], U4[:, 2 * NSB : 3 * NSB], U4[:, 3 * NSB : 4 * NSB], AL.mult
            )
            nc.vector.tensor_tensor(msk[:], Ux[:], Uy[:], AL.mult)
            nc.vector.tensor_scalar(pm[:], msk[:], -1000.0, 1000.0, AL.mult, AL.add)
            nc.vector.tensor_tensor(spx[:], SC[:, 0:NSB], pm[:], AL.add)
            nc.vector.tensor_tensor(spy[:], SC[:, NSB : 2 * NSB], pm[:], AL.add)
            nc.vector.tensor_scalar(zx[:], spx[:], rh[:, 0:1], None, AL.mult)
            nc.vector.tensor_scalar(zy[:], spy[:], rh[:, 1:2], None, AL.mult)

            # ---- eval grid GP = h*c (unscaled coords) ----
            nc.vector.tensor_scalar(
                GP[:, 0:G], IOT[:, 0:G], h[:, 0:1], None, AL.mult
            )
            nc.vector.tensor_scalar(
                GP[:, G : 2 * G], IOT[:, G : 2 * G], h[:, 1:2], None, AL.mult
            )

            # ---- Qt[v, j] = exp(-a'_y (gy_v - ly_j)^2)  [128, 1024] ----
            nc.gpsimd.partition_broadcast(LYB[:, :], LYR[0:1, :], 128)
            nc.vector.tensor_scalar(QD[:], LYB[:], gqc[:, 0:1], None, AL.subtract)
            nc.vector.scalar_tensor_tensor(
                QS[:], QD[:], na[:, 1:2], QD[:], AL.mult, AL.mult
            )
            nc.scalar.activation(Qt[:], QS[:], AF.Exp)

            # ---- binning: W[i, u|v] windows, Ht += Wy^T Wx  (PE bf16) ----
            for k in range(NSB):
                eng = nc.vector
                D = wapool.tile([128, 2 * G], f32, tag="wd")
                SQ = wapool.tile([128, 2 * G], f32, tag="wsq")
                eng.tensor_scalar(
                    D[:, 0:G], IOT[:, 0:G], zx[:, k : k + 1], None,
                    AL.subtract,
                )
                eng.tensor_scalar(
                    D[:, G : 2 * G], IOT[:, G : 2 * G], zy[:, k : k + 1], None,
                    AL.subtract,
                )
                eng.scalar_tensor_tensor(SQ[:], D[:], -0.5, D[:], AL.mult, AL.mult)
                W = wepool.tile([128, 2 * G], f32, tag="we")
                nc.scalar.activation(W[:], SQ[:], AF.Exp)
                nc.tensor.matmul(
                    Ht_ps[:, :],
                    lhsT=W[:, G : 2 * G],
                    rhs=W[:, 0:G],
                    start=(k == 0),
                    stop=(k == NSB - 1),
                )

            # ---- P/Q eval tiles + T2 += Q^T P  (location shard) ----
            for q in range(NLB):
                eng = nc.vector
                D = wapool.tile([128, 2 * G], f32, tag="wd")
                SQ = wapool.tile([128, 2 * G], f32, tag="wsq")
                eng.tensor_scalar(
                    D[:, 0:G], GP[:, 0:G], LXC[:, q : q + 1], None, AL.subtract
                )
                eng.tensor_scalar(
                    D[:, G : 2 * G], GP[:, G : 2 * G], LYC[:, q : q + 1], None,
                    AL.subtract,
                )
                eng.scalar_tensor_tensor(
                    SQ[:, 0:G], D[:, 0:G], na[:, 0:1], D[:, 0:G], AL.mult, AL.mult
                )
                eng.scalar_tensor_tensor(
                    SQ[:, G : 2 * G], D[:, G : 2 * G], na[:, 1:2], D[:, G : 2 * G],
                    AL.mult, AL.mult,
                )
                nc.scalar.activation(PQE[q][:], SQ[:], AF.Exp)
                nc.tensor.matmul(
                    T2_ps[:, :],
                    lhsT=PQE[q][:, G : 2 * G],
                    rhs=PQE[q][:, 0:G],
                    start=(q == 0),
                    stop=(q == NLB - 1),
                )

            # ---- pack + single AllReduce of [Ht | T2] ----
            nc.vector.tensor_copy(CCS[:, 0:G], Ht_ps[:, :])
            nc.scalar.copy(CCS[:, G : 2 * G], T2_ps[:, :])
            nc.sync.dma_start(out=cc_in[:, :], in_=CCS[:, :])
            nc.gpsimd.collective_compute(
                "AllReduce",
                AL.add,
                replica_groups=[list(range(N_CORES))],
                ins=[cc_in[:, :]],
                outs=[cc_out[:, :]],
            )
            nc.sync.dma_start(out=HTg[:, :], in_=cc_out[:, :])

            # ---- R[j,u] = sum_v Qt[v,j] * Ht[v,u]; out_j = sum_u P*R ----
            nc.vector.tensor_copy(Hb[:, :], HTg[:, 0:G])
            for q in range(NLB):
                nc.tensor.matmul(
                    R_ps[:, q * G : (q + 1) * G],
                    lhsT=Qt[:, q * G : (q + 1) * G],
                    rhs=Hb[:, :],
                    start=True,
                    stop=True,
                )
                nc.vector.scalar_tensor_tensor(
                    scr[:, :],
                    PQE[q][:, 0:G],
                    1.0,
                    R_ps[:, q * G : (q + 1) * G],
                    AL.mult,
                    AL.mult,
                    accum_out=ACC[:, q : q + 1],
                )

            # ---- norm = sum_{v,u} Ht*T2 ; out /= norm ----
            nc.vector.scalar_tensor_tensor(
                scr2[:, :],
                HTg[:, 0:G],
                1.0,
                HTg[:, G : 2 * G],
                AL.mult,
                AL.mult,
                accum_out=ns_[:, :],
            )
            nc.gpsimd.memset(ONEC[:, :], 1.0)
            nc.gpsimd.memset(ONER[:, :], 1.0)
            tot_ps = ppool.tile([1, 1], f32, tag="tot")
            rb_ps = ppool.tile([128, 1], f32, tag="rb")
            nc.tensor.matmul(
                tot_ps[:, :], lhsT=ns_[:, :], rhs=ONEC[:, :], start=True, stop=True
            )
            nc.vector.reciprocal(rtot_sb[:], tot_ps[:, :])
            nc.tensor.matmul(
                rb_ps[:, :], lhsT=ONER[:, :], rhs=rtot_sb[:, :], start=True, stop=True
            )
            nc.vector.tensor_copy(rb_sb[:, :], rb_ps[:, :])
            nc.vector.tensor_scalar(OUT[:], ACC[:], rb_sb[:, 0:1], None, AL.mult)
            nc.sync.dma_start(out=out_d[:, :], in_=OUT[:])

    nc.compile()
    return nc


def make_in_maps(samples, locations):
    sx = samples[:, 0].reshape(N_CORES, NSB, 128)
    sy = samples[:, 1].reshape(N_CORES, NSB, 128)
    lx = locations[:, 0].reshape(N_CORES, NLB, 128)
    ly = locations[:, 1].reshape(N_CORES, NLB, 128)
    la = np.concatenate(
        [
            np.ascontiguousarray(locations[:, 0].reshape(64, 128).T),
            np.ascontiguousarray(locations[:, 1].reshape(64, 128).T),
        ],
        axis=1,
    ).astype(np.float32)
    c = (np.arange(G, dtype=np.float32) - 63.5)
    iota_cb = np.tile(np.concatenate([c, c])[None, :], (128, 1))
    colc = (np.arange(128, dtype=np.float32) - 63.5)[:, None]
    in_maps = []
    for cid in range(N_CORES):
        s_cols = np.concatenate(
            [sx[cid].T, sy[cid].T], axis=1
        )  # [128, 32]: col k = block k
        l_xcols = lx[cid].T  # [128, 8] col q part m = shard[q*128+m]
        l_ycols = ly[cid].T
        l_yrow = ly[cid].reshape(1, NL_SH)
        in_maps.append(
            {
                "s_cols": np.ascontiguousarray(s_cols, dtype=np.float32),
                "l_xcols": np.ascontiguousarray(l_xcols, dtype=np.float32),
                "l_ycols": np.ascontiguousarray(l_ycols, dtype=np.float32),
                "l_yrow": np.ascontiguousarray(l_yrow, dtype=np.float32),
                "l_all": np.ascontiguousarray(la, dtype=np.float32),
                "iota_cb": np.ascontiguousarray(iota_cb, dtype=np.float32),
                "colc": np.ascontiguousarray(colc, dtype=np.float32),
            }
        )
    return in_maps


def kernel(samples, locations):
    samples = np.ascontiguousarray(np.asarray(samples, dtype=np.float32))
    locations = np.ascontiguousarray(np.asarray(locations, dtype=np.float32))
    assert samples.shape == (NS, 2) and locations.shape == (NL, 2)

    from concourse.bass_utils import run_bass_kernel_spmd

    if "nc" not in _STATE:
        _STATE["nc"] = build_nc()
    nc = _STATE["nc"]

    in_maps = make_in_maps(samples, locations)
    res = run_bass_kernel_spmd(
        nc,
        in_maps,
        list(range(N_CORES)),
        trace=bool(_STATE.get("trace", False)),
    )
    _STATE["exec_time_ns"] = res.exec_time_ns
    _STATE["profile_json"] = res.profile_json
    outs = [
        np.asarray(res.results[c]["out"], dtype=np.float32).T.reshape(NL_SH)
        for c in range(N_CORES)
    ]
    return np.concatenate(outs)


# revision 17
# speedup vs baseline: 1.0216x; 1.0216x over previous
"""Gaussian KDE (bandwidth=0.5) on 8 TRN2 NeuronCores — grid-factorized.

out[j] = sum_i mask_i * exp(-|s_i - l_j|^2 / bw^2), normalized to sum 1.

Algorithm (exact Gaussian-lattice factorization, NOT an approximation knob):
  exp(-|s-l|^2/(2v)) with v = bw^2/2 = 0.125 factorizes over a uniform grid
  g_u = h*c_u (c_u = u-63.5, h = 2M/119, M = per-axis abs-max of locations):

      sum_u exp(-(s-g_u)^2/(2h^2)) * exp(-(g_u-l)^2/(2v'))
        = C * exp(-(s-l)^2/(2(v'+h^2)))        [Gaussian o Gaussian, exact]
  with v' = v - h^2.  The lattice-sum constant C is independent of s up to
  a Poisson ripple exp(-2 pi^2) ~ 5e-9, and cancels in the normalization.

  So per core (samples sharded 8-way, locations sharded 8-way):
    Wx[i,u] = exp(-(sx_i-g_u)^2/(2h^2))   (x-window), same Wy     [2048 x 128]
    Ht[v,u] = sum_i Wy[i,v]*Wx[i,u]       (PE, partial over sample shard)
    P[j,u]  = exp(-a'(gx_u-lx_j)^2), Q[j,v] = exp(-a'(gy_v-ly_j)^2),
              a' = 1/(2 v')               (location shard, 1024 locs)
    T2[v,u] = sum_j Q[j,v]*P[j,u]         (PE, partial over location shard)
    ONE AllReduce of [Ht | T2]  (128x256 f32)
    R[j,u]  = sum_v Qt[v,j]*Ht[v,u]       (PE)
    out[j]  = sum_u P[j,u]*R[j,u],  norm = sum_{v,u} Ht*T2  (= sum_j out_j)
    out /= norm  (on device)

  Samples outside the location bbox (strict |s| < M per axis, torch mask
  semantics) are pushed +1000 before binning -> their window underflows to 0.

Engine plan: ScalarE runs ONLY Exp (no act-table switches); DVE+GpSimd build
the quadratic exp arguments with tensor_scalar/scalar_tensor_tensor; PE does
the three contractions in bf16 (operands are exps in [0,1]; rel err ~1e-3).
"""

import sys

sys.path.insert(0, "/opt/trn_rl_repo")

import numpy as np

N_CORES = 8
NS = 16384
NL = 8192
NS_SH = NS // N_CORES  # 2048 samples per core
NL_SH = NL // N_CORES  # 1024 locations per core
G = 128  # grid nodes per axis
NSB = NS_SH // 128  # 16 sample blocks
NLB = NL_SH // 128  # 8 location blocks
GDEN = 119.0  # grid half-width = M * 127/119ish margin (4h pad for windows)
V = 0.125  # bw^2 / 2

_STATE = {}


def build_nc():
    import concourse.bacc as bacc
    import concourse.mybir as mybir
    import concourse.tile as tile
    from concourse import bass_isa

    f32 = mybir.dt.float32
    bf16 = mybir.dt.bfloat16
    AX = mybir.AxisListType
    AF = mybir.ActivationFunctionType
    AL = mybir.AluOpType
    RO = bass_isa.ReduceOp

    nc = bacc.Bacc(None, target_bir_lowering=False, num_devices=N_CORES)

    s_cols = nc.declare_dram_parameter("s_cols", [128, 2 * NSB], f32, isOutput=False)
    l_xc = nc.declare_dram_parameter("l_xcols", [128, NLB], f32, isOutput=False)
    l_yc = nc.declare_dram_parameter("l_ycols", [128, NLB], f32, isOutput=False)
    l_yr = nc.declare_dram_parameter("l_yrow", [1, NL_SH], f32, isOutput=False)
    l_all = nc.declare_dram_parameter("l_all", [128, 128], f32, isOutput=False)
    iot_d = nc.declare_dram_parameter("iota_cb", [128, 2 * G], f32, isOutput=False)
    col_d = nc.declare_dram_parameter("colc", [128, 1], f32, isOutput=False)
    out_d = nc.declare_dram_parameter("out", [128, NLB], f32, isOutput=True)

    with tile.TileContext(nc) as tc:
        with tc.tile_pool(name="const", bufs=1) as cpool, \
             tc.tile_pool(name="dram", bufs=1, space="DRAM") as dpool, \
             tc.tile_pool(name="wa", bufs=3) as wapool, \
             tc.tile_pool(name="wexp", bufs=4) as wepool, \
             tc.tile_pool(name="ps", bufs=1, space="PSUM") as ppool:

            SC = cpool.tile([128, 2 * NSB], f32)  # sample cols [sx | sy]
            LXC = cpool.tile([128, NLB], f32)
            LYC = cpool.tile([128, NLB], f32)
            LYR = cpool.tile([1, NL_SH], f32)
            LA = cpool.tile([128, 128], f32)
            IOT = cpool.tile([128, 2 * G], f32)  # c_u both halves
            COLC = cpool.tile([128, 1], f32)  # c_p per partition

            rm = cpool.tile([128, 2], f32)
            Mb = cpool.tile([128, 2], f32)
            h = cpool.tile([128, 2], f32)
            rh = cpool.tile([128, 2], f32)
            hsq = cpool.tile([128, 2], f32)
            vp = cpool.tile([128, 2], f32)
            rvp = cpool.tile([128, 2], f32)
            na = cpool.tile([128, 2], f32)  # -a' per axis
            gqc = cpool.tile([128, 1], f32)  # gy_v = h_y * c_v

            nSC = cpool.tile([128, 2 * NSB], f32)
            U4 = cpool.tile([128, 4 * NSB], f32)
            Ux = cpool.tile([128, NSB], f32)
            Uy = cpool.tile([128, NSB], f32)
            msk = cpool.tile([128, NSB], f32)
            pm = cpool.tile([128, NSB], f32)
            spx = cpool.tile([128, NSB], f32)
            spy = cpool.tile([128, NSB], f32)
            zx = cpool.tile([128, NSB], f32)
            zy = cpool.tile([128, NSB], f32)

            GP = cpool.tile([128, 2 * G], f32)  # [gx_u | gy_u]
            LYB = cpool.tile([128, NL_SH], f32)
            QD = cpool.tile([128, NL_SH], f32)
            QS = cpool.tile([128, NL_SH], f32)
            Qt = cpool.tile([128, NL_SH], bf16)
            PQE = [cpool.tile([128, 2 * G], f32, name=f"pqe{q}") for q in range(NLB)]

            CCS = cpool.tile([128, 2 * G], f32)
            Hb = cpool.tile([128, G], bf16)
            ONEC = cpool.tile([128, 1], f32)
            ONER = cpool.tile([1, 128], f32)
            rtot_sb = cpool.tile([1, 1], f32)
            rb_sb = cpool.tile([128, 1], f32)
            HTg = cpool.tile([128, 2 * G], f32)
            ACC = cpool.tile([128, NLB], f32)
            scr = cpool.tile([128, G], f32)
            scr2 = cpool.tile([128, G], f32)
            ns_ = cpool.tile([128, 1], f32)
            ntb = cpool.tile([128, 1], f32)
            rtot = cpool.tile([128, 1], f32)
            OUT = cpool.tile([128, NLB], f32)

            cc_in = dpool.tile([128, 2 * G], f32, name="cc_in")
            cc_out = dpool.tile([128, 2 * G], f32, addr_space="Shared", name="cc_out")

            Ht_ps = ppool.tile([128, G], f32, tag="ht")
            T2_ps = ppool.tile([128, G], f32, tag="t2")
            R_ps = ppool.tile([128, NL_SH], f32, tag="r")

            # ---- input loads ----
            nc.sync.dma_start(out=SC[:, :], in_=s_cols[:, :])
            nc.sync.dma_start(out=LXC[:, :], in_=l_xc[:, :])
            nc.sync.dma_start(out=LYC[:, :], in_=l_yc[:, :])
            nc.sync.dma_start(out=LYR[:, :], in_=l_yr[:, :])
            nc.sync.dma_start(out=LA[:, :], in_=l_all[:, :])
            nc.sync.dma_start(out=IOT[:, :], in_=iot_d[:, :])
            nc.sync.dma_start(out=COLC[:, :], in_=col_d[:, :])

            # ---- bbox bounds M (global over all 8192 locations) ----
            nc.vector.tensor_reduce(
                rm[:, 0:1], LA[:, 0:64], axis=AX.X, op=AL.max,
                apply_absolute_value=True,
            )
            nc.vector.tensor_reduce(
                rm[:, 1:2], LA[:, 64:128], axis=AX.X, op=AL.max,
                apply_absolute_value=True,
            )
            nc.gpsimd.partition_all_reduce(Mb[:, :], rm[:, :], 128, RO.max)

            # ---- runtime scalars (all [128,2] broadcast, x col 0 / y col 1) ----
            nc.vector.tensor_scalar(h[:], Mb[:], 2.0 / GDEN, None, AL.mult)
            nc.vector.reciprocal(rh[:], h[:])
            nc.vector.tensor_tensor(hsq[:], h[:], h[:], AL.mult)
            nc.vector.tensor_scalar(vp[:], hsq[:], -1.0, V, AL.mult, AL.add)
            nc.vector.reciprocal(rvp[:], vp[:])
            nc.vector.tensor_scalar(na[:], rvp[:], -0.5, None, AL.mult)
            nc.vector.tensor_scalar(gqc[:], COLC[:], h[:, 1:2], None, AL.mult)

            # ---- sample prep: mask + z = s/h  ([128, NSB] col k = block) ----
            nc.vector.tensor_scalar(nSC[:], SC[:], -1.0, None, AL.mult)
            nc.vector.tensor_scalar(
                U4[:, 0:NSB], SC[:, 0:NSB], Mb[:, 0:1], None, AL.is_lt
            )
            nc.vector.tensor_scalar(
                U4[:, NSB : 2 * NSB], nSC[:, 0:NSB], Mb[:, 0:1], None, AL.is_lt
            )
            nc.vector.tensor_scalar(
                U4[:, 2 * NSB : 3 * NSB], SC[:, NSB : 2 * NSB], Mb[:, 1:2], None,
                AL.is_lt,
            )
            nc.vector.tensor_scalar(
                U4[:, 3 * NSB : 4 * NSB], nSC[:, NSB : 2 * NSB], Mb[:, 1:2], None,
                AL.is_lt,
            )
            nc.vector.tensor_tensor(
                Ux[:], U4[:, 0:NSB], U4[:, NSB : 2 * NSB], AL.mult
            )
            nc.vector.tensor_tensor(
                Uy[:], U4[:, 2 * NSB : 3 * NSB], U4[:, 3 * NSB : 4 * NSB], AL.mult
            )
            nc.vector.tensor_tensor(msk[:], Ux[:], Uy[:], AL.mult)
            nc.vector.tensor_scalar(pm[:], msk[:], -1000.0, 1000.0, AL.mult, AL.add)
            nc.vector.tensor_tensor(spx[:], SC[:, 0:NSB], pm[:], AL.add)
            nc.vector.tensor_tensor(spy[:], SC[:, NSB : 2 * NSB], pm[:], AL.add)
            nc.vector.tensor_scalar(zx[:], spx[:], rh[:, 0:1], None, AL.mult)
            nc.vector.tensor_scalar(zy[:], spy[:], rh[:, 1:2], None, AL.mult)

            # ---- eval grid GP = h*c (unscaled coords) ----
            nc.vector.tensor_scalar(
                GP[:, 0:G], IOT[:, 0:G], h[:, 0:1], None, AL.mult
            )
            nc.vector.tensor_scalar(
                GP[:, G : 2 * G], IOT[:, G : 2 * G], h[:, 1:2], None, AL.mult
            )

            # ---- Qt[v, j] = exp(-a'_y (gy_v - ly_j)^2)  [128, 1024] ----
            nc.gpsimd.partition_broadcast(LYB[:, :], LYR[0:1, :], 128)
            nc.vector.tensor_scalar(QD[:], LYB[:], gqc[:, 0:1], None, AL.subtract)
            nc.vector.scalar_tensor_tensor(
                QS[:], QD[:], na[:, 1:2], QD[:], AL.mult, AL.mult
            )
            nc.scalar.activation(Qt[:], QS[:], AF.Exp)

            # ---- binning: W[i, u|v] windows, Ht += Wy^T Wx  (PE bf16) ----
            for k in range(NSB):
                eng = nc.vector
                D = wapool.tile([128, 2 * G], f32, tag="wd")
                SQ = wapool.tile([128, 2 * G], f32, tag="wsq")
                eng.tensor_scalar(
                    D[:, 0:G], IOT[:, 0:G], zx[:, k : k + 1], None,
                    AL.subtract,
                )
                eng.tensor_scalar(
                    D[:, G : 2 * G], IOT[:, G : 2 * G], zy[:, k : k + 1], None,
                    AL.subtract,
                )
                eng.scalar_tensor_tensor(SQ[:], D[:], -0.5, D[:], AL.mult, AL.mult)
                W = wepool.tile([128, 2 * G], f32, tag="we")
                nc.scalar.activation(W[:], SQ[:], AF.Exp)
                nc.tensor.matmul(
                    Ht_ps[:, :],
                    lhsT=W[:, G : 2 * G],
                    rhs=W[:, 0:G],
                    start=(k == 0),
                    stop=(k == NSB - 1),
                )

            # ---- P/Q eval tiles + T2 += Q^T P  (location shard) ----
            for q in range(NLB):
                eng = nc.vector
                D = wapool.tile([128, 2 * G], f32, tag="wd")
                SQ = wapool.tile([128, 2 * G], f32, tag="wsq")
                eng.tensor_scalar(
                    D[:, 0:G], GP[:, 0:G], LXC[:, q : q + 1], None, AL.subtract
                )
                eng.tensor_scalar(
                    D[:, G : 2 * G], GP[:, G : 2 * G], LYC[:, q : q + 1], None,
                    AL.subtract,
                )
                eng.scalar_tensor_tensor(
                    SQ[:, 0:G], D[:, 0:G], na[:, 0:1], D[:, 0:G], AL.mult, AL.mult
                )
                eng.scalar_tensor_tensor(
                    SQ[:, G : 2 * G], D[:, G : 2 * G], na[:, 1:2], D[:, G : 2 * G],
                    AL.mult, AL.mult,
                )
                nc.scalar.activation(PQE[q][:], SQ[:], AF.Exp)
                nc.tensor.matmul(
                    T2_ps[:, :],
                    lhsT=PQE[q][:, G : 2 * G],
                    rhs=PQE[q][:, 0:G],
                    start=(q == 0),
                    stop=(q == NLB - 1),
                )

            # ---- pack + single AllReduce of [Ht | T2] ----
            nc.vector.tensor_copy(CCS[:, 0:G], Ht_ps[:, :])
            nc.scalar.copy(CCS[:, G : 2 * G], T2_ps[:, :])
            nc.sync.dma_start(out=cc_in[:, :], in_=CCS[:, :])
            nc.gpsimd.collective_compute(
                "AllReduce",
                AL.add,
                replica_groups=[list(range(N_CORES))],
                ins=[cc_in[:, :]],
                outs=[cc_out[:, :]],
            )
            nc.sync.dma_start(out=HTg[:, :], in_=cc_out[:, :])

            # ---- R[j,u] = sum_v Qt[v,j] * Ht[v,u]; out_j = sum_u P*R ----
            nc.vector.tensor_copy(Hb[:, :], HTg[:, 0:G])
            for q in range(NLB):
                nc.tensor.matmul(
                    R_ps[:, q * G : (q + 1) * G],
                    lhsT=Qt[:, q * G : (q + 1) * G],
                    rhs=Hb[:, :],
                    start=True,
                    stop=True,
                )
                nc.vector.scalar_tensor_tensor(
                    scr[:, :],
                    PQE[q][:, 0:G],
                    1.0,
                    R_ps[:, q * G : (q + 1) * G],
                    AL.mult,
                    AL.mult,
                    accum_out=ACC[:, q : q + 1],
                )

            # ---- norm = sum_{v,u} Ht*T2 ; out /= norm ----
            nc.vector.scalar_tensor_tensor(
                scr2[:, :],
                HTg[:, 0:G],
                1.0,
                HTg[:, G : 2 * G],
                AL.mult,
                AL.mult,
                accum_out=ns_[:, :],
            )
            nc.gpsimd.memset(ONEC[:, :], 1.0)
            nc.gpsimd.memset(ONER[:, :], 1.0)
            tot_ps = ppool.tile([1, 1], f32, tag="tot")
            rb_ps = ppool.tile([128, 1], f32, tag="rb")
            nc.tensor.matmul(
                tot_ps[:, :], lhsT=ns_[:, :], rhs=ONEC[:, :], start=True, stop=True
            )
            nc.vector.reciprocal(rtot_sb[:], tot_ps[:, :])
            nc.tensor.matmul(
                rb_ps[:, :], lhsT=ONER[:, :], rhs=rtot_sb[:, :], start=True, stop=True
            )
            nc.vector.tensor_copy(rb_sb[:, :], rb_ps[:, :])
            nc.vector.tensor_scalar(OUT[:], ACC[:], rb_sb[:, 0:1], None, AL.mult)
            nc.sync.dma_start(out=out_d[:, :], in_=OUT[:])

    nc.compile()
    return nc


def make_in_maps(samples, locations):
    sx = samples[:, 0].reshape(N_CORES, NSB, 128)
    sy = samples[:, 1].reshape(N_CORES, NSB, 128)
    lx = locations[:, 0].reshape(N_CORES, NLB, 128)
    ly = locations[:, 1].reshape(N_CORES, NLB, 128)
    la = np.concatenate(
        [
            np.ascontiguousarray(locations[:, 0].reshape(64, 128).T),
            np.ascontiguousarray(locations[:, 1].reshape(64, 128).T),
        ],
        axis=1,
    ).astype(np.float32)
    c = (np.arange(G, dtype=np.float32) - 63.5)
    iota_cb = np.tile(np.concatenate([c, c])[None, :], (128, 1))
    colc = (np.arange(128, dtype=np.float32) - 63.5)[:, None]
    in_maps = []
    for cid in range(N_CORES):
        s_cols = np.concatenate(
            [sx[cid].T, sy[cid].T], axis=1
        )  # [128, 32]: col k = block k
        l_xcols = lx[cid].T  # [128, 8] col q part m = shard[q*128+m]
        l_ycols = ly[cid].T
        l_yrow = ly[cid].reshape(1, NL_SH)
        in_maps.append(
            {
                "s_cols": np.ascontiguousarray(s_cols, dtype=np.float32),
                "l_xcols": np.ascontiguousarray(l_xcols, dtype=np.float32),
                "l_ycols": np.ascontiguousarray(l_ycols, dtype=np.float32),
                "l_yrow": np.ascontiguousarray(l_yrow, dtype=np.float32),
                "l_all": np.ascontiguousarray(la, dtype=np.float32),
                "iota_cb": np.ascontiguousarray(iota_cb, dtype=np.float32),
                "colc": np.ascontiguousarray(colc, dtype=np.float32),
            }
        )
    return in_maps


def kernel(samples, locations):
    samples = np.ascontiguousarray(np.asarray(samples, dtype=np.float32))
    locations = np.ascontiguousarray(np.asarray(locations, dtype=np.float32))
    assert samples.shape == (NS, 2) and locations.shape == (NL, 2)

    from concourse.bass_utils import run_bass_kernel_spmd

    if "nc" not in _STATE:
        _STATE["nc"] = build_nc()
    nc = _STATE["nc"]

    in_maps = make_in_maps(samples, locations)
    res = run_bass_kernel_spmd(
        nc,
        in_maps,
        list(range(N_CORES)),
        trace=bool(_STATE.get("trace", False)),
    )
    _STATE["exec_time_ns"] = res.exec_time_ns
    _STATE["profile_json"] = res.profile_json
    outs = [
        np.asarray(res.results[c]["out"], dtype=np.float32).T.reshape(NL_SH)
        for c in range(N_CORES)
    ]
    return np.concatenate(outs)


# revision 18
# speedup vs baseline: 1.0821x; 1.0592x over previous
"""Gaussian KDE (bandwidth=0.5) on 8 TRN2 NeuronCores — grid-factorized.

out[j] = sum_i mask_i * exp(-|s_i - l_j|^2 / bw^2), normalized to sum 1.

Algorithm (exact Gaussian-lattice factorization, NOT an approximation knob):
  exp(-|s-l|^2/(2v)) with v = bw^2/2 = 0.125 factorizes over a uniform grid
  g_u = h*c_u (c_u = u-63.5, h = 2M/119, M = per-axis abs-max of locations):

      sum_u exp(-(s-g_u)^2/(2h^2)) * exp(-(g_u-l)^2/(2v'))
        = C * exp(-(s-l)^2/(2(v'+h^2)))        [Gaussian o Gaussian, exact]
  with v' = v - h^2.  The lattice-sum constant C is independent of s up to
  a Poisson ripple exp(-2 pi^2) ~ 5e-9, and cancels in the normalization.

  So per core (samples sharded 8-way, locations sharded 8-way):
    Wx[i,u] = exp(-(sx_i-g_u)^2/(2h^2))   (x-window), same Wy     [2048 x 128]
    Ht[v,u] = sum_i Wy[i,v]*Wx[i,u]       (PE, partial over sample shard)
    P[j,u]  = exp(-a'(gx_u-lx_j)^2), Q[j,v] = exp(-a'(gy_v-ly_j)^2),
              a' = 1/(2 v')               (location shard, 1024 locs)
    T2[v,u] = sum_j Q[j,v]*P[j,u]         (PE, partial over location shard)
    ONE AllReduce of [Ht | T2]  (128x256 f32)
    R[j,u]  = sum_v Qt[v,j]*Ht[v,u]       (PE)
    out[j]  = sum_u P[j,u]*R[j,u],  norm = sum_{v,u} Ht*T2  (= sum_j out_j)
    out /= norm  (on device)

  Samples outside the location bbox (strict |s| < M per axis, torch mask
  semantics) are pushed +1000 before binning -> their window underflows to 0.

Engine plan: ScalarE runs ONLY Exp (no act-table switches); DVE+GpSimd build
the quadratic exp arguments with tensor_scalar/scalar_tensor_tensor; PE does
the three contractions in bf16 (operands are exps in [0,1]; rel err ~1e-3).
"""

import sys

sys.path.insert(0, "/opt/trn_rl_repo")

import numpy as np

N_CORES = 8
NS = 16384
NL = 8192
NS_SH = NS // N_CORES  # 2048 samples per core
NL_SH = NL // N_CORES  # 1024 locations per core
G = 128  # grid nodes per axis
NSB = NS_SH // 128  # 16 sample blocks
NLB = NL_SH // 128  # 8 location blocks
GDEN = 119.0  # grid half-width = M * 127/119ish margin (4h pad for windows)
V = 0.125  # bw^2 / 2

_STATE = {}


def build_nc():
    import concourse.bacc as bacc
    import concourse.mybir as mybir
    import concourse.tile as tile
    from concourse import bass_isa

    f32 = mybir.dt.float32
    bf16 = mybir.dt.bfloat16
    AX = mybir.AxisListType
    AF = mybir.ActivationFunctionType
    AL = mybir.AluOpType
    RO = bass_isa.ReduceOp

    nc = bacc.Bacc(None, target_bir_lowering=False, num_devices=N_CORES)

    s_cols = nc.declare_dram_parameter("s_cols", [128, 2 * NSB], f32, isOutput=False)
    l_xc = nc.declare_dram_parameter("l_xcols", [128, NLB], f32, isOutput=False)
    l_yc = nc.declare_dram_parameter("l_ycols", [128, NLB], f32, isOutput=False)
    l_yr = nc.declare_dram_parameter("l_yrow", [1, NL_SH], f32, isOutput=False)
    l_all = nc.declare_dram_parameter("l_all", [128, 128], f32, isOutput=False)
    iot_d = nc.declare_dram_parameter("iota_cb", [128, 2 * G], f32, isOutput=False)
    col_d = nc.declare_dram_parameter("colc", [128, 1], f32, isOutput=False)
    out_d = nc.declare_dram_parameter("out", [128, NLB], f32, isOutput=True)

    with tile.TileContext(nc) as tc:
        with tc.tile_pool(name="const", bufs=1) as cpool, \
             tc.tile_pool(name="dram", bufs=1, space="DRAM") as dpool, \
             tc.tile_pool(name="wa", bufs=3) as wapool, \
             tc.tile_pool(name="wexp", bufs=4) as wepool, \
             tc.tile_pool(name="ps", bufs=1, space="PSUM") as ppool:

            SC = cpool.tile([128, 2 * NSB], f32)  # sample cols [sx | sy]
            LXC = cpool.tile([128, NLB], f32)
            LYC = cpool.tile([128, NLB], f32)
            LYR = cpool.tile([1, NL_SH], f32)
            LA = cpool.tile([128, 128], f32)
            IOT = cpool.tile([128, 2 * G], f32)  # c_u both halves
            COLC = cpool.tile([128, 1], f32)  # c_p per partition

            rm = cpool.tile([128, 2], f32)
            Mb = cpool.tile([128, 2], f32)
            h = cpool.tile([128, 2], f32)
            rh = cpool.tile([128, 2], f32)
            hsq = cpool.tile([128, 2], f32)
            vp = cpool.tile([128, 2], f32)
            rvp = cpool.tile([128, 2], f32)
            na = cpool.tile([128, 2], f32)  # -a' per axis
            gqc = cpool.tile([128, 1], f32)  # gy_v = h_y * c_v

            nSC = cpool.tile([128, 2 * NSB], f32)
            U4 = cpool.tile([128, 4 * NSB], f32)
            Ux = cpool.tile([128, NSB], f32)
            Uy = cpool.tile([128, NSB], f32)
            msk = cpool.tile([128, NSB], f32)
            pm = cpool.tile([128, NSB], f32)
            spx = cpool.tile([128, NSB], f32)
            spy = cpool.tile([128, NSB], f32)
            zx = cpool.tile([128, NSB], f32)
            zy = cpool.tile([128, NSB], f32)

            GP = cpool.tile([128, 2 * G], f32)  # [gx_u | gy_u]
            LYB = cpool.tile([128, NL_SH], f32)
            QD = cpool.tile([128, NL_SH], f32)
            QS = cpool.tile([128, NL_SH], f32)
            Qt = cpool.tile([128, NL_SH], bf16)
            PQE = [cpool.tile([128, 2 * G], f32, name=f"pqe{q}") for q in range(NLB)]

            CCS = cpool.tile([128, 2 * G], bf16)
            ONEC = cpool.tile([128, 1], f32)
            ONER = cpool.tile([1, 128], f32)
            rtot_sb = cpool.tile([1, 1], f32)
            rb_sb = cpool.tile([128, 1], f32)
            HTg = cpool.tile([128, 2 * G], bf16)
            ACC = cpool.tile([128, NLB], f32)
            scr = cpool.tile([128, G], f32)
            scr2 = cpool.tile([128, G], f32)
            ns_ = cpool.tile([128, 1], f32)
            ntb = cpool.tile([128, 1], f32)
            rtot = cpool.tile([128, 1], f32)
            OUT = cpool.tile([128, NLB], f32)

            cc_in = dpool.tile([128, 2 * G], bf16, name="cc_in")
            cc_out = dpool.tile([128, 2 * G], bf16, addr_space="Shared", name="cc_out")

            Ht_ps = ppool.tile([128, G], f32, tag="ht")
            T2_ps = ppool.tile([128, G], f32, tag="t2")
            R_ps = ppool.tile([128, NL_SH], f32, tag="r")

            # ---- input loads ----
            nc.sync.dma_start(out=SC[:, :], in_=s_cols[:, :])
            nc.sync.dma_start(out=LXC[:, :], in_=l_xc[:, :])
            nc.sync.dma_start(out=LYC[:, :], in_=l_yc[:, :])
            nc.sync.dma_start(out=LYR[:, :], in_=l_yr[:, :])
            nc.sync.dma_start(out=LA[:, :], in_=l_all[:, :])
            nc.sync.dma_start(out=IOT[:, :], in_=iot_d[:, :])
            nc.sync.dma_start(out=COLC[:, :], in_=col_d[:, :])

            # ---- bbox bounds M (global over all 8192 locations) ----
            nc.vector.tensor_reduce(
                rm[:, 0:1], LA[:, 0:64], axis=AX.X, op=AL.max,
                apply_absolute_value=True,
            )
            nc.vector.tensor_reduce(
                rm[:, 1:2], LA[:, 64:128], axis=AX.X, op=AL.max,
                apply_absolute_value=True,
            )
            nc.gpsimd.partition_all_reduce(Mb[:, :], rm[:, :], 128, RO.max)

            # ---- runtime scalars (all [128,2] broadcast, x col 0 / y col 1) ----
            nc.vector.tensor_scalar(h[:], Mb[:], 2.0 / GDEN, None, AL.mult)
            nc.vector.reciprocal(rh[:], h[:])
            nc.vector.tensor_tensor(hsq[:], h[:], h[:], AL.mult)
            nc.vector.tensor_scalar(vp[:], hsq[:], -1.0, V, AL.mult, AL.add)
            nc.vector.reciprocal(rvp[:], vp[:])
            nc.vector.tensor_scalar(na[:], rvp[:], -0.5, None, AL.mult)
            nc.vector.tensor_scalar(gqc[:], COLC[:], h[:, 1:2], None, AL.mult)

            # ---- sample prep: mask + z = s/h  ([128, NSB] col k = block) ----
            nc.vector.tensor_scalar(nSC[:], SC[:], -1.0, None, AL.mult)
            nc.vector.tensor_scalar(
                U4[:, 0:NSB], SC[:, 0:NSB], Mb[:, 0:1], None, AL.is_lt
            )
            nc.vector.tensor_scalar(
                U4[:, NSB : 2 * NSB], nSC[:, 0:NSB], Mb[:, 0:1], None, AL.is_lt
            )
            nc.vector.tensor_scalar(
                U4[:, 2 * NSB : 3 * NSB], SC[:, NSB : 2 * NSB], Mb[:, 1:2], None,
                AL.is_lt,
            )
            nc.vector.tensor_scalar(
                U4[:, 3 * NSB : 4 * NSB], nSC[:, NSB : 2 * NSB], Mb[:, 1:2], None,
                AL.is_lt,
            )
            nc.vector.tensor_tensor(
                Ux[:], U4[:, 0:NSB], U4[:, NSB : 2 * NSB], AL.mult
            )
            nc.vector.tensor_tensor(
                Uy[:], U4[:, 2 * NSB : 3 * NSB], U4[:, 3 * NSB : 4 * NSB], AL.mult
            )
            nc.vector.tensor_tensor(msk[:], Ux[:], Uy[:], AL.mult)
            nc.vector.tensor_scalar(pm[:], msk[:], -1000.0, 1000.0, AL.mult, AL.add)
            nc.vector.tensor_tensor(spx[:], SC[:, 0:NSB], pm[:], AL.add)
            nc.vector.tensor_tensor(spy[:], SC[:, NSB : 2 * NSB], pm[:], AL.add)
            nc.vector.tensor_scalar(zx[:], spx[:], rh[:, 0:1], None, AL.mult)
            nc.vector.tensor_scalar(zy[:], spy[:], rh[:, 1:2], None, AL.mult)

            # ---- eval grid GP = h*c (unscaled coords) ----
            nc.vector.tensor_scalar(
                GP[:, 0:G], IOT[:, 0:G], h[:, 0:1], None, AL.mult
            )
            nc.vector.tensor_scalar(
                GP[:, G : 2 * G], IOT[:, G : 2 * G], h[:, 1:2], None, AL.mult
            )

            # ---- Qt[v, j] = exp(-a'_y (gy_v - ly_j)^2)  [128, 1024] ----
            nc.gpsimd.partition_broadcast(LYB[:, :], LYR[0:1, :], 128)
            nc.vector.tensor_scalar(QD[:], LYB[:], gqc[:, 0:1], None, AL.subtract)
            nc.vector.scalar_tensor_tensor(
                QS[:], QD[:], na[:, 1:2], QD[:], AL.mult, AL.mult
            )
            nc.scalar.activation(Qt[:], QS[:], AF.Exp)

            # ---- binning: W[i, u|v] windows, Ht += Wy^T Wx  (PE bf16) ----
            for k in range(NSB):
                eng = nc.vector
                D = wapool.tile([128, 2 * G], f32, tag="wd")
                SQ = wapool.tile([128, 2 * G], f32, tag="wsq")
                eng.tensor_scalar(
                    D[:, 0:G], IOT[:, 0:G], zx[:, k : k + 1], None,
                    AL.subtract,
                )
                eng.tensor_scalar(
                    D[:, G : 2 * G], IOT[:, G : 2 * G], zy[:, k : k + 1], None,
                    AL.subtract,
                )
                eng.scalar_tensor_tensor(SQ[:], D[:], -0.5, D[:], AL.mult, AL.mult)
                W = wepool.tile([128, 2 * G], f32, tag="we")
                nc.scalar.activation(W[:], SQ[:], AF.Exp)
                nc.tensor.matmul(
                    Ht_ps[:, :],
                    lhsT=W[:, G : 2 * G],
                    rhs=W[:, 0:G],
                    start=(k == 0),
                    stop=(k == NSB - 1),
                )

            # ---- P/Q eval tiles + T2 += Q^T P  (location shard) ----
            for q in range(NLB):
                eng = nc.vector
                D = wapool.tile([128, 2 * G], f32, tag="wd")
                SQ = wapool.tile([128, 2 * G], f32, tag="wsq")
                eng.tensor_scalar(
                    D[:, 0:G], GP[:, 0:G], LXC[:, q : q + 1], None, AL.subtract
                )
                eng.tensor_scalar(
                    D[:, G : 2 * G], GP[:, G : 2 * G], LYC[:, q : q + 1], None,
                    AL.subtract,
                )
                eng.scalar_tensor_tensor(
                    SQ[:, 0:G], D[:, 0:G], na[:, 0:1], D[:, 0:G], AL.mult, AL.mult
                )
                eng.scalar_tensor_tensor(
                    SQ[:, G : 2 * G], D[:, G : 2 * G], na[:, 1:2], D[:, G : 2 * G],
                    AL.mult, AL.mult,
                )
                nc.scalar.activation(PQE[q][:], SQ[:], AF.Exp)
                nc.tensor.matmul(
                    T2_ps[:, :],
                    lhsT=PQE[q][:, G : 2 * G],
                    rhs=PQE[q][:, 0:G],
                    start=(q == 0),
                    stop=(q == NLB - 1),
                )

            # ---- pack + single AllReduce of [Ht | T2] ----
            nc.vector.tensor_copy(CCS[:, 0:G], Ht_ps[:, :])
            nc.scalar.copy(CCS[:, G : 2 * G], T2_ps[:, :])
            nc.sync.dma_start(out=cc_in[:, :], in_=CCS[:, :])
            nc.gpsimd.collective_compute(
                "AllReduce",
                AL.add,
                replica_groups=[list(range(N_CORES))],
                ins=[cc_in[:, :]],
                outs=[cc_out[:, :]],
            )
            nc.sync.dma_start(out=HTg[:, :], in_=cc_out[:, :])

            # ---- R[j,u] = sum_v Qt[v,j] * Ht[v,u]; out_j = sum_u P*R ----
            # ---- norm = sum_{v,u} Ht*T2 -> rb = 1/norm (broadcast) ----
            nc.gpsimd.memset(ONEC[:, :], 1.0)
            nc.gpsimd.memset(ONER[:, :], 1.0)
            nc.vector.scalar_tensor_tensor(
                scr2[:, :],
                HTg[:, 0:G],
                1.0,
                HTg[:, G : 2 * G],
                AL.mult,
                AL.mult,
                accum_out=ns_[:, :],
            )
            tot_ps = ppool.tile([1, 1], f32, tag="tot")
            rb_ps = ppool.tile([128, 1], f32, tag="rb")
            nc.tensor.matmul(
                tot_ps[:, :], lhsT=ns_[:, :], rhs=ONEC[:, :], start=True, stop=True
            )
            nc.vector.reciprocal(rtot_sb[:], tot_ps[:, :])
            nc.tensor.matmul(
                rb_ps[:, :], lhsT=ONER[:, :], rhs=rtot_sb[:, :], start=True, stop=True
            )
            nc.vector.tensor_copy(rb_sb[:, :], rb_ps[:, :])

            # ---- R[j,u] = sum_v Qt[v,j]*Ht[v,u]; out_j = sum_u (P*rb)*R ----
            for q in range(NLB):
                nc.tensor.matmul(
                    R_ps[:, q * G : (q + 1) * G],
                    lhsT=Qt[:, q * G : (q + 1) * G],
                    rhs=HTg[:, 0:G],
                    start=True,
                    stop=True,
                )
                nc.vector.scalar_tensor_tensor(
                    scr[:, :],
                    PQE[q][:, 0:G],
                    rb_sb[:, 0:1],
                    R_ps[:, q * G : (q + 1) * G],
                    AL.mult,
                    AL.mult,
                    accum_out=ACC[:, q : q + 1],
                )
            nc.sync.dma_start(out=out_d[:, :], in_=ACC[:])

    nc.compile()
    return nc


def make_in_maps(samples, locations):
    sx = samples[:, 0].reshape(N_CORES, NSB, 128)
    sy = samples[:, 1].reshape(N_CORES, NSB, 128)
    lx = locations[:, 0].reshape(N_CORES, NLB, 128)
    ly = locations[:, 1].reshape(N_CORES, NLB, 128)
    la = np.concatenate(
        [
            np.ascontiguousarray(locations[:, 0].reshape(64, 128).T),
            np.ascontiguousarray(locations[:, 1].reshape(64, 128).T),
        ],
        axis=1,
    ).astype(np.float32)
    c = (np.arange(G, dtype=np.float32) - 63.5)
    iota_cb = np.tile(np.concatenate([c, c])[None, :], (128, 1))
    colc = (np.arange(128, dtype=np.float32) - 63.5)[:, None]
    in_maps = []
    for cid in range(N_CORES):
        s_cols = np.concatenate(
            [sx[cid].T, sy[cid].T], axis=1
        )  # [128, 32]: col k = block k
        l_xcols = lx[cid].T  # [128, 8] col q part m = shard[q*128+m]
        l_ycols = ly[cid].T
        l_yrow = ly[cid].reshape(1, NL_SH)
        in_maps.append(
            {
                "s_cols": np.ascontiguousarray(s_cols, dtype=np.float32),
                "l_xcols": np.ascontiguousarray(l_xcols, dtype=np.float32),
                "l_ycols": np.ascontiguousarray(l_ycols, dtype=np.float32),
                "l_yrow": np.ascontiguousarray(l_yrow, dtype=np.float32),
                "l_all": np.ascontiguousarray(la, dtype=np.float32),
                "iota_cb": np.ascontiguousarray(iota_cb, dtype=np.float32),
                "colc": np.ascontiguousarray(colc, dtype=np.float32),
            }
        )
    return in_maps


def kernel(samples, locations):
    samples = np.ascontiguousarray(np.asarray(samples, dtype=np.float32))
    locations = np.ascontiguousarray(np.asarray(locations, dtype=np.float32))
    assert samples.shape == (NS, 2) and locations.shape == (NL, 2)

    from concourse.bass_utils import run_bass_kernel_spmd

    if "nc" not in _STATE:
        _STATE["nc"] = build_nc()
    nc = _STATE["nc"]

    in_maps = make_in_maps(samples, locations)
    res = run_bass_kernel_spmd(
        nc,
        in_maps,
        list(range(N_CORES)),
        trace=bool(_STATE.get("trace", False)),
    )
    _STATE["exec_time_ns"] = res.exec_time_ns
    _STATE["profile_json"] = res.profile_json
    outs = [
        np.asarray(res.results[c]["out"], dtype=np.float32).T.reshape(NL_SH)
        for c in range(N_CORES)
    ]
    return np.concatenate(outs)


# revision 19
# speedup vs baseline: 1.1242x; 1.0389x over previous
"""Gaussian KDE (bandwidth=0.5) on 8 TRN2 NeuronCores — grid-factorized.

out[j] = sum_i mask_i * exp(-|s_i - l_j|^2 / bw^2), normalized to sum 1.

Algorithm (exact Gaussian-lattice factorization, NOT an approximation knob):
  exp(-|s-l|^2/(2v)) with v = bw^2/2 = 0.125 factorizes over a uniform grid
  g_u = h*c_u (c_u = u-63.5, h = 2M/119, M = per-axis abs-max of locations):

      sum_u exp(-(s-g_u)^2/(2h^2)) * exp(-(g_u-l)^2/(2v'))
        = C * exp(-(s-l)^2/(2(v'+h^2)))        [Gaussian o Gaussian, exact]
  with v' = v - h^2.  The lattice-sum constant C is independent of s up to
  a Poisson ripple exp(-2 pi^2) ~ 5e-9, and cancels in the normalization.

  So per core (samples sharded 8-way, locations sharded 8-way):
    Wx[i,u] = exp(-(sx_i-g_u)^2/(2h^2))   (x-window), same Wy     [2048 x 128]
    Ht[v,u] = sum_i Wy[i,v]*Wx[i,u]       (PE, partial over sample shard)
    P[j,u]  = exp(-a'(gx_u-lx_j)^2), Q[j,v] = exp(-a'(gy_v-ly_j)^2),
              a' = 1/(2 v')               (location shard, 1024 locs)
    T2[v,u] = sum_j Q[j,v]*P[j,u]         (PE, partial over location shard)
    ONE AllReduce of [Ht | T2]  (128x256 f32)
    R[j,u]  = sum_v Qt[v,j]*Ht[v,u]       (PE)
    out[j]  = sum_u P[j,u]*R[j,u],  norm = sum_{v,u} Ht*T2  (= sum_j out_j)
    out /= norm  (on device)

  Samples outside the location bbox (strict |s| < M per axis, torch mask
  semantics) are pushed +1000 before binning -> their window underflows to 0.

Engine plan: ScalarE runs ONLY Exp (no act-table switches); DVE+GpSimd build
the quadratic exp arguments with tensor_scalar/scalar_tensor_tensor; PE does
the three contractions in bf16 (operands are exps in [0,1]; rel err ~1e-3).
"""

import sys

sys.path.insert(0, "/opt/trn_rl_repo")

import numpy as np

N_CORES = 8
NS = 16384
NL = 8192
NS_SH = NS // N_CORES  # 2048 samples per core
NL_SH = NL // N_CORES  # 1024 locations per core
G = 128  # grid nodes per axis
NSB = NS_SH // 128  # 16 sample blocks
NLB = NL_SH // 128  # 8 location blocks
GDEN = 119.0  # grid half-width = M * 127/119ish margin (4h pad for windows)
V = 0.125  # bw^2 / 2

_STATE = {}


def build_nc():
    import concourse.bacc as bacc
    import concourse.mybir as mybir
    import concourse.tile as tile
    from concourse import bass_isa

    f32 = mybir.dt.float32
    bf16 = mybir.dt.bfloat16
    AX = mybir.AxisListType
    AF = mybir.ActivationFunctionType
    AL = mybir.AluOpType
    RO = bass_isa.ReduceOp

    nc = bacc.Bacc(None, target_bir_lowering=False, num_devices=N_CORES)

    s_cols = nc.declare_dram_parameter("s_cols", [128, 2 * NSB], f32, isOutput=False)
    l_xc = nc.declare_dram_parameter("l_xcols", [128, NLB], f32, isOutput=False)
    l_yc = nc.declare_dram_parameter("l_ycols", [128, NLB], f32, isOutput=False)
    l_yr = nc.declare_dram_parameter("l_yrow", [1, NL_SH], f32, isOutput=False)
    l_all = nc.declare_dram_parameter("l_all", [128, 128], f32, isOutput=False)
    iot_d = nc.declare_dram_parameter("iota_cb", [128, 2 * G], f32, isOutput=False)
    col_d = nc.declare_dram_parameter("colc", [128, 1], f32, isOutput=False)
    out_d = nc.declare_dram_parameter("out", [128, NLB], f32, isOutput=True)

    with tile.TileContext(nc) as tc:
        with tc.tile_pool(name="const", bufs=1) as cpool, \
             tc.tile_pool(name="dram", bufs=1, space="DRAM") as dpool, \
             tc.tile_pool(name="wa", bufs=3) as wapool, \
             tc.tile_pool(name="wexp", bufs=4) as wepool, \
             tc.tile_pool(name="ps", bufs=1, space="PSUM") as ppool:

            SC = cpool.tile([128, 2 * NSB], f32)  # sample cols [sx | sy]
            LXC = cpool.tile([128, NLB], f32)
            LYC = cpool.tile([128, NLB], f32)
            LYR = cpool.tile([1, NL_SH], f32)
            LA = cpool.tile([128, 128], f32)
            IOT = cpool.tile([128, 2 * G], f32)  # c_u both halves
            COLC = cpool.tile([128, 1], f32)  # c_p per partition

            rm = cpool.tile([128, 2], f32)
            Mb = cpool.tile([128, 2], f32)
            h = cpool.tile([128, 2], f32)
            rh = cpool.tile([128, 2], f32)
            hsq = cpool.tile([128, 2], f32)
            vp = cpool.tile([128, 2], f32)
            rvp = cpool.tile([128, 2], f32)
            na = cpool.tile([128, 2], f32)  # -a' per axis
            gqc = cpool.tile([128, 1], f32)  # gy_v = h_y * c_v

            nSC = cpool.tile([128, 2 * NSB], f32)
            U4 = cpool.tile([128, 4 * NSB], f32)
            Ux = cpool.tile([128, NSB], f32)
            Uy = cpool.tile([128, NSB], f32)
            msk = cpool.tile([128, NSB], f32)
            pm = cpool.tile([128, NSB], f32)
            spx = cpool.tile([128, NSB], f32)
            spy = cpool.tile([128, NSB], f32)
            zx = cpool.tile([128, NSB], f32)
            zy = cpool.tile([128, NSB], f32)

            GP = cpool.tile([128, 2 * G], f32)  # [gx_u | gy_u]
            LYB = cpool.tile([128, NL_SH], f32)
            QD = cpool.tile([128, NL_SH], f32)
            QS = cpool.tile([128, NL_SH], f32)
            Qt = cpool.tile([128, NL_SH], bf16)
            PQE = [cpool.tile([128, 2 * G], f32, name=f"pqe{q}") for q in range(NLB)]

            CCS = cpool.tile([128, 2 * G], bf16)
            ONEC = cpool.tile([128, 1], f32)
            ONER = cpool.tile([1, 128], f32)
            rtot_sb = cpool.tile([1, 1], f32)
            rb_sb = cpool.tile([128, 1], f32)
            HTg = cpool.tile([128, 2 * G], bf16)
            ACC = cpool.tile([128, NLB], f32)
            scr = cpool.tile([128, G], f32)
            scr2 = cpool.tile([128, G], f32)
            ns_ = cpool.tile([128, 1], f32)
            ntb = cpool.tile([128, 1], f32)
            rtot = cpool.tile([128, 1], f32)
            OUT = cpool.tile([128, NLB], f32)

            cc_in = dpool.tile([128, 2 * G], bf16, name="cc_in")
            cc_out = dpool.tile([8 * 128, 2 * G], bf16, addr_space="Shared", name="cc_out")

            Ht_ps = ppool.tile([128, G], f32, tag="ht")
            T2_ps = ppool.tile([128, G], f32, tag="t2")
            R_ps = ppool.tile([128, NL_SH], f32, tag="r")

            # ---- input loads ----
            nc.sync.dma_start(out=SC[:, :], in_=s_cols[:, :])
            nc.sync.dma_start(out=LXC[:, :], in_=l_xc[:, :])
            nc.sync.dma_start(out=LYC[:, :], in_=l_yc[:, :])
            nc.sync.dma_start(out=LYR[:, :], in_=l_yr[:, :])
            nc.sync.dma_start(out=LA[:, :], in_=l_all[:, :])
            nc.sync.dma_start(out=IOT[:, :], in_=iot_d[:, :])
            nc.sync.dma_start(out=COLC[:, :], in_=col_d[:, :])

            # ---- bbox bounds M (global over all 8192 locations) ----
            nc.vector.tensor_reduce(
                rm[:, 0:1], LA[:, 0:64], axis=AX.X, op=AL.max,
                apply_absolute_value=True,
            )
            nc.vector.tensor_reduce(
                rm[:, 1:2], LA[:, 64:128], axis=AX.X, op=AL.max,
                apply_absolute_value=True,
            )
            nc.gpsimd.partition_all_reduce(Mb[:, :], rm[:, :], 128, RO.max)

            # ---- runtime scalars (all [128,2] broadcast, x col 0 / y col 1) ----
            nc.vector.tensor_scalar(h[:], Mb[:], 2.0 / GDEN, None, AL.mult)
            nc.vector.reciprocal(rh[:], h[:])
            nc.vector.tensor_tensor(hsq[:], h[:], h[:], AL.mult)
            nc.vector.tensor_scalar(vp[:], hsq[:], -1.0, V, AL.mult, AL.add)
            nc.vector.reciprocal(rvp[:], vp[:])
            nc.vector.tensor_scalar(na[:], rvp[:], -0.5, None, AL.mult)
            nc.vector.tensor_scalar(gqc[:], COLC[:], h[:, 1:2], None, AL.mult)

            # ---- sample prep: mask + z = s/h  ([128, NSB] col k = block) ----
            nc.vector.tensor_scalar(nSC[:], SC[:], -1.0, None, AL.mult)
            nc.vector.tensor_scalar(
                U4[:, 0:NSB], SC[:, 0:NSB], Mb[:, 0:1], None, AL.is_lt
            )
            nc.vector.tensor_scalar(
                U4[:, NSB : 2 * NSB], nSC[:, 0:NSB], Mb[:, 0:1], None, AL.is_lt
            )
            nc.vector.tensor_scalar(
                U4[:, 2 * NSB : 3 * NSB], SC[:, NSB : 2 * NSB], Mb[:, 1:2], None,
                AL.is_lt,
            )
            nc.vector.tensor_scalar(
                U4[:, 3 * NSB : 4 * NSB], nSC[:, NSB : 2 * NSB], Mb[:, 1:2], None,
                AL.is_lt,
            )
            nc.vector.tensor_tensor(
                Ux[:], U4[:, 0:NSB], U4[:, NSB : 2 * NSB], AL.mult
            )
            nc.vector.tensor_tensor(
                Uy[:], U4[:, 2 * NSB : 3 * NSB], U4[:, 3 * NSB : 4 * NSB], AL.mult
            )
            nc.vector.tensor_tensor(msk[:], Ux[:], Uy[:], AL.mult)
            nc.vector.tensor_scalar(pm[:], msk[:], -1000.0, 1000.0, AL.mult, AL.add)
            nc.vector.tensor_tensor(spx[:], SC[:, 0:NSB], pm[:], AL.add)
            nc.vector.tensor_tensor(spy[:], SC[:, NSB : 2 * NSB], pm[:], AL.add)
            nc.vector.tensor_scalar(zx[:], spx[:], rh[:, 0:1], None, AL.mult)
            nc.vector.tensor_scalar(zy[:], spy[:], rh[:, 1:2], None, AL.mult)

            # ---- eval grid GP = h*c (unscaled coords) ----
            nc.vector.tensor_scalar(
                GP[:, 0:G], IOT[:, 0:G], h[:, 0:1], None, AL.mult
            )
            nc.vector.tensor_scalar(
                GP[:, G : 2 * G], IOT[:, G : 2 * G], h[:, 1:2], None, AL.mult
            )

            # ---- Qt[v, j] = exp(-a'_y (gy_v - ly_j)^2)  [128, 1024] ----
            nc.gpsimd.partition_broadcast(LYB[:, :], LYR[0:1, :], 128)
            nc.vector.tensor_scalar(QD[:], LYB[:], gqc[:, 0:1], None, AL.subtract)
            nc.vector.scalar_tensor_tensor(
                QS[:], QD[:], na[:, 1:2], QD[:], AL.mult, AL.mult
            )
            nc.scalar.activation(Qt[:], QS[:], AF.Exp)

            # ---- binning: W[i, u|v] windows, Ht += Wy^T Wx  (PE bf16) ----
            for k in range(NSB):
                eng = nc.vector
                D = wapool.tile([128, 2 * G], f32, tag="wd")
                SQ = wapool.tile([128, 2 * G], f32, tag="wsq")
                eng.tensor_scalar(
                    D[:, 0:G], IOT[:, 0:G], zx[:, k : k + 1], None,
                    AL.subtract,
                )
                eng.tensor_scalar(
                    D[:, G : 2 * G], IOT[:, G : 2 * G], zy[:, k : k + 1], None,
                    AL.subtract,
                )
                eng.scalar_tensor_tensor(SQ[:], D[:], -0.5, D[:], AL.mult, AL.mult)
                W = wepool.tile([128, 2 * G], f32, tag="we")
                nc.scalar.activation(W[:], SQ[:], AF.Exp)
                nc.tensor.matmul(
                    Ht_ps[:, :],
                    lhsT=W[:, G : 2 * G],
                    rhs=W[:, 0:G],
                    start=(k == 0),
                    stop=(k == NSB - 1),
                )

            # ---- P/Q eval tiles + T2 += Q^T P  (location shard) ----
            for q in range(NLB):
                eng = nc.vector
                D = wapool.tile([128, 2 * G], f32, tag="wd")
                SQ = wapool.tile([128, 2 * G], f32, tag="wsq")
                eng.tensor_scalar(
                    D[:, 0:G], GP[:, 0:G], LXC[:, q : q + 1], None, AL.subtract
                )
                eng.tensor_scalar(
                    D[:, G : 2 * G], GP[:, G : 2 * G], LYC[:, q : q + 1], None,
                    AL.subtract,
                )
                eng.scalar_tensor_tensor(
                    SQ[:, 0:G], D[:, 0:G], na[:, 0:1], D[:, 0:G], AL.mult, AL.mult
                )
                eng.scalar_tensor_tensor(
                    SQ[:, G : 2 * G], D[:, G : 2 * G], na[:, 1:2], D[:, G : 2 * G],
                    AL.mult, AL.mult,
                )
                nc.scalar.activation(PQE[q][:], SQ[:], AF.Exp)
                nc.tensor.matmul(
                    T2_ps[:, :],
                    lhsT=PQE[q][:, G : 2 * G],
                    rhs=PQE[q][:, 0:G],
                    start=(q == 0),
                    stop=(q == NLB - 1),
                )

            # ---- pack + single AllReduce of [Ht | T2] ----
            nc.vector.tensor_copy(CCS[:, 0:G], Ht_ps[:, :])
            nc.scalar.copy(CCS[:, G : 2 * G], T2_ps[:, :])
            nc.sync.dma_start(out=cc_in[:, :], in_=CCS[:, :])
            nc.gpsimd.collective_compute(
                "AllGather",
                AL.bypass,
                replica_groups=[list(range(N_CORES))],
                ins=[cc_in[:, :]],
                outs=[cc_out[:, :]],
            )
            GATH = cpool.tile([128, 8 * 2 * G], bf16)
            for c in range(N_CORES):
                nc.sync.dma_start(
                    out=GATH[:, c * 2 * G : (c + 1) * 2 * G],
                    in_=cc_out[c * 128 : (c + 1) * 128, :],
                )
            # pairwise tree sum of the 8 partials (bf16 ins, f32 accum)
            L1 = [cpool.tile([128, 2 * G], f32, name=f"l1_{i}") for i in range(4)]
            L2 = [cpool.tile([128, 2 * G], f32, name=f"l2_{i}") for i in range(2)]
            HTf = cpool.tile([128, 2 * G], f32)
            for i in range(4):
                nc.vector.tensor_tensor(
                    L1[i][:, :],
                    GATH[:, (2 * i) * 2 * G : (2 * i + 1) * 2 * G],
                    GATH[:, (2 * i + 1) * 2 * G : (2 * i + 2) * 2 * G],
                    AL.add,
                )
            nc.vector.tensor_tensor(L2[0][:, :], L1[0][:, :], L1[1][:, :], AL.add)
            nc.vector.tensor_tensor(L2[1][:, :], L1[2][:, :], L1[3][:, :], AL.add)
            nc.vector.tensor_tensor(HTf[:, :], L2[0][:, :], L2[1][:, :], AL.add)
            nc.vector.tensor_copy(HTg[:, :], HTf[:, :])

            # ---- R[j,u] = sum_v Qt[v,j] * Ht[v,u]; out_j = sum_u P*R ----
            # ---- norm = sum_{v,u} Ht*T2 -> rb = 1/norm (broadcast) ----
            nc.gpsimd.memset(ONEC[:, :], 1.0)
            nc.gpsimd.memset(ONER[:, :], 1.0)
            nc.vector.scalar_tensor_tensor(
                scr2[:, :],
                HTg[:, 0:G],
                1.0,
                HTg[:, G : 2 * G],
                AL.mult,
                AL.mult,
                accum_out=ns_[:, :],
            )
            tot_ps = ppool.tile([1, 1], f32, tag="tot")
            rb_ps = ppool.tile([128, 1], f32, tag="rb")
            nc.tensor.matmul(
                tot_ps[:, :], lhsT=ns_[:, :], rhs=ONEC[:, :], start=True, stop=True
            )
            nc.vector.reciprocal(rtot_sb[:], tot_ps[:, :])
            nc.tensor.matmul(
                rb_ps[:, :], lhsT=ONER[:, :], rhs=rtot_sb[:, :], start=True, stop=True
            )
            nc.vector.tensor_copy(rb_sb[:, :], rb_ps[:, :])

            # ---- R[j,u] = sum_v Qt[v,j]*Ht[v,u]; out_j = sum_u (P*rb)*R ----
            for q in range(NLB):
                nc.tensor.matmul(
                    R_ps[:, q * G : (q + 1) * G],
                    lhsT=Qt[:, q * G : (q + 1) * G],
                    rhs=HTg[:, 0:G],
                    start=True,
                    stop=True,
                )
                nc.vector.scalar_tensor_tensor(
                    scr[:, :],
                    PQE[q][:, 0:G],
                    rb_sb[:, 0:1],
                    R_ps[:, q * G : (q + 1) * G],
                    AL.mult,
                    AL.mult,
                    accum_out=ACC[:, q : q + 1],
                )
            nc.sync.dma_start(out=out_d[:, :], in_=ACC[:])

    nc.compile()
    return nc


def make_in_maps(samples, locations):
    sx = samples[:, 0].reshape(N_CORES, NSB, 128)
    sy = samples[:, 1].reshape(N_CORES, NSB, 128)
    lx = locations[:, 0].reshape(N_CORES, NLB, 128)
    ly = locations[:, 1].reshape(N_CORES, NLB, 128)
    la = np.concatenate(
        [
            np.ascontiguousarray(locations[:, 0].reshape(64, 128).T),
            np.ascontiguousarray(locations[:, 1].reshape(64, 128).T),
        ],
        axis=1,
    ).astype(np.float32)
    c = (np.arange(G, dtype=np.float32) - 63.5)
    iota_cb = np.tile(np.concatenate([c, c])[None, :], (128, 1))
    colc = (np.arange(128, dtype=np.float32) - 63.5)[:, None]
    in_maps = []
    for cid in range(N_CORES):
        s_cols = np.concatenate(
            [sx[cid].T, sy[cid].T], axis=1
        )  # [128, 32]: col k = block k
        l_xcols = lx[cid].T  # [128, 8] col q part m = shard[q*128+m]
        l_ycols = ly[cid].T
        l_yrow = ly[cid].reshape(1, NL_SH)
        in_maps.append(
            {
                "s_cols": np.ascontiguousarray(s_cols, dtype=np.float32),
                "l_xcols": np.ascontiguousarray(l_xcols, dtype=np.float32),
                "l_ycols": np.ascontiguousarray(l_ycols, dtype=np.float32),
                "l_yrow": np.ascontiguousarray(l_yrow, dtype=np.float32),
                "l_all": np.ascontiguousarray(la, dtype=np.float32),
                "iota_cb": np.ascontiguousarray(iota_cb, dtype=np.float32),
                "colc": np.ascontiguousarray(colc, dtype=np.float32),
            }
        )
    return in_maps


def kernel(samples, locations):
    samples = np.ascontiguousarray(np.asarray(samples, dtype=np.float32))
    locations = np.ascontiguousarray(np.asarray(locations, dtype=np.float32))
    assert samples.shape == (NS, 2) and locations.shape == (NL, 2)

    from concourse.bass_utils import run_bass_kernel_spmd

    if "nc" not in _STATE:
        _STATE["nc"] = build_nc()
    nc = _STATE["nc"]

    in_maps = make_in_maps(samples, locations)
    res = run_bass_kernel_spmd(
        nc,
        in_maps,
        list(range(N_CORES)),
        trace=bool(_STATE.get("trace", False)),
    )
    _STATE["exec_time_ns"] = res.exec_time_ns
    _STATE["profile_json"] = res.profile_json
    outs = [
        np.asarray(res.results[c]["out"], dtype=np.float32).T.reshape(NL_SH)
        for c in range(N_CORES)
    ]
    return np.concatenate(outs)
